# revision 8
# baseline (speedup 1.0000x reference)
"""Trainium2 Bass kernel for CheckpointedIntegratedMTAPFM (LSTM enc/dec + PFM).

Self-contained: hardcodes shapes, shards batch B=16 across 8 NeuronCores
(2 per core), builds one Bass program (SPMD), runs via
bass_utils.run_bass_kernel_spmd, and reassembles full outputs.

Device algorithm (per core, Bc=2, NB=4096 sequences, halves of 2048):
  - Unified 32-step recurrence (20 encoder + 12 decoder steps) with the
    input embedding fused into the layer-1 LSTM matmul (K=66 via
    Wih@emb_w), per-gate packed [128,2048] tiles (two batch halves on
    partitions), and double-buffered recurrent tiles
    RH_A=[h1A;h2A], RH_B=[h2B;h1B] so layer 2 runs as single K=128
    matmuls per (gate, half) using row-swapped weight copies.
  - Coefficients head after encoder; autoregressive position decoding
    with scalar_tensor_tensor accumulation.
  - Potential-field integration in an [ (d,e) x (b,a) ] layout with
    pairwise diffs / force-sums done as tiny matmuls on the PE.
"""
import sys
import types

sys.path.insert(0, "/opt/trn_rl_repo")

import numpy as np

import concourse.bass as bass
import concourse.mybir as mybir
import concourse.tile as tile
from concourse.vector_clock import ScopedClock

F32 = np.float32
DT32 = mybir.dt.float32
Alu = mybir.AluOpType
Act = mybir.ActivationFunctionType

# shapes (hardcoded per problem spec)
B, A, E, H, D = 16, 128, 16, 20, 2
HID = 64
T, PL = 20, 12
NCORES = 8
BC = B // NCORES          # 2
NB = BC * A * E           # 4096
HALF = NB // 2            # 2048
BA = BC * A               # 256
NCH = HALF // 512         # 4 free chunks of 512 per half

DT = 0.1
TAS = 4.087
MIN_SP = F32(TAS * (1 - 0.15))
MAX_SP = F32(TAS * (1 + 0.15))
TH_HI = float(F32((F32(0.5) - F32(1e-6)) * (F32(0.5) - F32(1e-6))))
TH_LO = float(F32((F32(1e-5) - F32(1e-6)) * (F32(1e-5) - F32(1e-6))))

MAX_WAITS = 1  # this walrus build rejects >1 sem-wait per instruction


def _install_ntff_hook():
    if "antenv.axon_hooks" in sys.modules:
        return
    try:
        if "/root/.axon_site" not in sys.path:
            sys.path.insert(0, "/root/.axon_site")
        from trn_agent_boot.trn_boot import _ntff_profile_via_ctypes

        hook = _ntff_profile_via_ctypes("/opt/axon/libaxon_pjrt.so")
    except Exception:
        hook = None
    mod = types.ModuleType("antenv.axon_hooks")
    mod._HOOK = hook
    mod.set_axon_ntff_profile_hook = lambda h: setattr(mod, "_HOOK", h)
    mod.get_axon_ntff_profile_hook = lambda: mod._HOOK
    sys.modules["antenv.axon_hooks"] = mod


class TileContextSplit(tile.TileContext):
    """Splits >MAX_WAITS sem-waits (ISA limit) onto NoOp carriers."""

    def _commit_instruction(self, inst, lazy_reg_writes: bool = True):
        si = inst.sync_info
        if (
            si is not None
            and si.on_wait
            and len(si.on_wait) > MAX_WAITS
            and not isinstance(inst, mybir.InstEventSemaphore)
        ):
            waits = list(si.on_wait)
            si.on_wait = waits[-MAX_WAITS:]
            for w in waits[:-MAX_WAITS]:
                carrier = mybir.InstNoOp(
                    name=self.nc.get_next_instruction_name(),
                    ins=[],
                    outs=[],
                    engine=inst.engine,
                    sync_info=mybir.SyncInfo(on_wait=[w], on_update=[]),
                )
                self._add_instruction(carrier)
        return super()._commit_instruction(inst, lazy_reg_writes)

    def _drain_and_barrier(self, tick_clock, wait_clock):
        drain_inst = self.nc.sync.drain()
        wait_clock.add_sem_waits(
            drain_inst.ins, ScopedClock({None: tick_clock.global_clock})
        )
        si = drain_inst.ins.sync_info
        if si is not None and si.on_wait and len(si.on_wait) > MAX_WAITS:
            waits = list(si.on_wait)
            si.on_wait = waits[:MAX_WAITS]
            rest = waits[MAX_WAITS:]
            while rest:
                chunk, rest = rest[:MAX_WAITS], rest[MAX_WAITS:]
                extra = self.nc.sync.drain()
                esi = extra.ins.sync_info
                if esi is None:
                    extra.ins.sync_info = mybir.SyncInfo(
                        on_wait=list(chunk), on_update=[]
                    )
                else:
                    esi.on_wait.extend(chunk)
        self.nc.all_engine_barrier()
        assert self.sems is not None
        popped = self.nc._tile_sem_poison_stack.pop()
        assert popped is self._sem_poison
        self.nc.clear_and_free_semaphores(list(self.sems.allocated().values()))
        self.nc.all_engine_barrier()


# ---------------------------------------------------------------------------
# device program
# ---------------------------------------------------------------------------

def _emit_gate_l1(nc, psum, Wh, Wx, rhA, rhB, xt, gcol):
    """layer-1 gate pre-activation: [128,2048] psum, halves packed."""
    gs = slice(gcol * HID, (gcol + 1) * HID)
    for c in range(NCH):
        f = slice(c * 512, (c + 1) * 512)
        # half A -> out rows 0:64 (cols 0-63), h at rows 0-63, x at rows 64-65
        if rhA is not None:
            nc.tensor.matmul(psum[0:64, f], Wh[0:64, gs], rhA[0:64, f],
                             start=True, stop=False, tile_position=(0, 0))
        nc.tensor.matmul(psum[0:64, f], Wx[64:66, gs], xt[64:66, f],
                         start=(rhA is None), stop=True, tile_position=(64, 0))
        # half B -> out rows 64:128 (cols 64-127), h rows 64-127, x rows 0-1
        f2 = slice(HALF + c * 512, HALF + (c + 1) * 512)
        if rhB is not None:
            nc.tensor.matmul(psum[64:128, f], Wh[64:128, gs], rhB[64:128, f],
                             start=True, stop=False, tile_position=(64, 64))
        nc.tensor.matmul(psum[64:128, f], Wx[0:2, gs], xt[0:2, f2],
                         start=(rhB is None), stop=True, tile_position=(0, 64))


def _emit_gate_l2(nc, psum, WA, WB, rhA, rhB, gcol, first_step):
    """layer-2 gate pre-activation, FLIPPED packing (half A at rows 64:128)."""
    gs = slice(gcol * HID, (gcol + 1) * HID)
    for c in range(NCH):
        f = slice(c * 512, (c + 1) * 512)
        if first_step:
            # h2=0: only the Wih2 @ h1 part.
            nc.tensor.matmul(psum[64:128, f], WA[0:64, gs], rhA[0:64, f],
                             start=True, stop=True, tile_position=(0, 64))
            nc.tensor.matmul(psum[0:64, f], WB[64:128, gs], rhB[64:128, f],
                             start=True, stop=True, tile_position=(64, 0))
        else:
            nc.tensor.matmul(psum[64:128, f], WA[:, gs], rhA[:, f],
                             start=True, stop=True, tile_position=(0, 64))
            nc.tensor.matmul(psum[0:64, f], WB[:, gs], rhB[:, f],
                             start=True, stop=True, tile_position=(0, 0))


def _build_program():
    nc = bass.Bass("TRN2")

    # --- DRAM tensors ---
    histT = nc.dram_tensor("histT", [2, T, NB], DT32, kind="ExternalInput")
    goal_d = nc.dram_tensor("goal_d", [2, BA], DT32, kind="ExternalInput")
    WL1 = nc.dram_tensor("WL1", [128, 256], DT32, kind="ExternalInput")
    WX1 = nc.dram_tensor("WX1", [66, 256], DT32, kind="ExternalInput")
    WL2A = nc.dram_tensor("WL2A", [128, 256], DT32, kind="ExternalInput")
    WL2B = nc.dram_tensor("WL2B", [128, 256], DT32, kind="ExternalInput")
    VL1 = nc.dram_tensor("VL1", [128, 256], DT32, kind="ExternalInput")
    VX1 = nc.dram_tensor("VX1", [66, 256], DT32, kind="ExternalInput")
    VL2A = nc.dram_tensor("VL2A", [128, 256], DT32, kind="ExternalInput")
    VL2B = nc.dram_tensor("VL2B", [128, 256], DT32, kind="ExternalInput")
    BE1 = nc.dram_tensor("BE1", [128, 4], DT32, kind="ExternalInput")
    BE2 = nc.dram_tensor("BE2", [128, 4], DT32, kind="ExternalInput")
    BD1 = nc.dram_tensor("BD1", [128, 4], DT32, kind="ExternalInput")
    BD2 = nc.dram_tensor("BD2", [128, 4], DT32, kind="ExternalInput")
    CT = nc.dram_tensor("CT", [128, 3], DT32, kind="ExternalInput")
    CB = nc.dram_tensor("CB", [3, 1], DT32, kind="ExternalInput")
    OT = nc.dram_tensor("OT", [128, 2], DT32, kind="ExternalInput")
    OB = nc.dram_tensor("OB", [66, 1], DT32, kind="ExternalInput")
    LDIF = nc.dram_tensor("LDIF", [32, 512], DT32, kind="ExternalInput")
    LSUM = nc.dram_tensor("LSUM", [128, 128], DT32, kind="ExternalInput")
    LREP = nc.dram_tensor("LREP", [16, 256], DT32, kind="ExternalInput")
    LNRM = nc.dram_tensor("LNRM", [32, 16], DT32, kind="ExternalInput")
    LDUP = nc.dram_tensor("LDUP", [16, 32], DT32, kind="ExternalInput")

    coeffs_o = nc.dram_tensor("coeffs_o", [3, NB], DT32, kind="ExternalOutput")
    decoded_o = nc.dram_tensor("decoded_o", [2, PL, NB], DT32,
                               kind="ExternalOutput")
    adjusted_o = nc.dram_tensor("adjusted_o", [PL, 32, BA], DT32,
                                kind="ExternalOutput")

    with TileContextSplit(nc) as tc:
        import contextlib
        with contextlib.ExitStack() as ctx:
            consts = ctx.enter_context(tc.tile_pool(name="consts", bufs=1))
            # LSTM-phase pools: closed before the PFM phase to free
            # SBUF/PSUM (stack allocator, LIFO).
            state_cm = tc.tile_pool(name="state", bufs=1)
            state = state_cm.__enter__()
            sig_cm = tc.tile_pool(name="sig", bufs=2)
            sig = sig_cm.__enter__()
            gp_cm = tc.tile_pool(name="gp", bufs=2, space="PSUM")
            gp = gp_cm.__enter__()

            def load_const(dram, shape):
                t_ = consts.tile(shape, DT32, tag=dram.name, name=dram.name + "_s")
                nc.sync.dma_start(out=t_[:, :], in_=dram[:, :])
                return t_

            WL1s = load_const(WL1, [128, 256])
            WX1s = load_const(WX1, [66, 256])
            WL2As = load_const(WL2A, [128, 256])
            WL2Bs = load_const(WL2B, [128, 256])
            VL1s = load_const(VL1, [128, 256])
            VX1s = load_const(VX1, [66, 256])
            VL2As = load_const(VL2A, [128, 256])
            VL2Bs = load_const(VL2B, [128, 256])
            BE1s = load_const(BE1, [128, 4])
            BE2s = load_const(BE2, [128, 4])
            BD1s = load_const(BD1, [128, 4])
            BD2s = load_const(BD2, [128, 4])
            CTs = load_const(CT, [128, 3])
            CBs = load_const(CB, [3, 1])
            OTs = load_const(OT, [128, 2])
            OBs = load_const(OB, [66, 1])
            LDIFs = load_const(LDIF, [32, 512])
            LSUMs = load_const(LSUM, [128, 128])
            LREPs = load_const(LREP, [16, 256])
            LNRMs = load_const(LNRM, [32, 16])
            LDUPs = load_const(LDUP, [16, 32])

            # recurrent state
            RH_A = [state.tile([128, HALF], DT32, tag=f"RHA{i}", name=f"RHA{i}") for i in range(2)]
            RH_B = [state.tile([128, HALF], DT32, tag=f"RHB{i}", name=f"RHB{i}") for i in range(2)]
            XT = [state.tile([66, NB], DT32, tag=f"XT{i}", name=f"XT{i}") for i in range(2)]
            c_t = [state.tile([128, HALF], DT32, tag=f"c{i}", name=f"ct{i}") for i in range(2)]

            coeffs_sb = state.tile([3, NB], DT32, tag="coeffs_sb", name="coeffs_sb")

            # ------------------- unified recurrence -------------------
            for u in range(T + PL):
                p, pn = u % 2, (u + 1) % 2
                enc = u < T
                Wh1, Wx1, W2A, W2B = (
                    (WL1s, WX1s, WL2As, WL2Bs) if enc
                    else (VL1s, VX1s, VL2As, VL2Bs))
                B1, B2 = (BE1s, BE2s) if enc else (BD1s, BD2s)

                # x input for this step
                if u <= T:
                    tsrc = u if u < T else T - 1  # u==T: last_pos
                    nc.sync.dma_start(out=XT[p][64:66, 0:HALF],
                                      in_=histT[:, tsrc, 0:HALF])
                    nc.sync.dma_start(out=XT[p][0:2, HALF:NB],
                                      in_=histT[:, tsrc, HALF:NB])
                # u>T: pred written by previous decoder step.

                for layer in (1, 2):
                    gates = []
                    for g in range(4):
                        psum = gp.tile([128, HALF], DT32, tag="gpsum", name="gpsum")
                        if layer == 1:
                            _emit_gate_l1(
                                nc, psum, Wh1, Wx1,
                                None if u == 0 else RH_A[pn],
                                None if u == 0 else RH_B[pn],
                                XT[p], g)
                        else:
                            _emit_gate_l2(nc, psum, W2A, W2B,
                                          RH_A[p], RH_B[p], g, u == 0)
                        act = sig.tile([128, HALF], DT32, tag=f"s{g}")
                        func = Act.Tanh if g == 2 else Act.Sigmoid
                        bias = (B1 if layer == 1 else B2)[:, g:g + 1]
                        nc.scalar.activation(act[:, :], psum[:, :], func,
                                             bias=bias)
                        gates.append(act)
                    si_, sf_, tg_, so_ = gates
                    cl = c_t[layer - 1]
                    if u == 0:
                        # c = sigmoid(i) * tanh(g)
                        nc.vector.tensor_mul(cl[:, :], si_[:, :], tg_[:, :])
                    else:
                        # t_ = si*tg (overwrite si), u_ = sf*c (gpsimd,
                        # overwrite sf), c = t_ + u_
                        nc.vector.tensor_mul(si_[:, :], si_[:, :], tg_[:, :])
                        nc.gpsimd.tensor_tensor(sf_[:, :], sf_[:, :], cl[:, :],
                                                Alu.mult)
                        nc.vector.tensor_add(cl[:, :], si_[:, :], sf_[:, :])
                    # tanh(c) -> reuse tg tile
                    nc.scalar.activation(tg_[:, :], cl[:, :], Act.Tanh)
                    if layer == 1:
                        nc.vector.tensor_mul(RH_A[p][0:64, :], so_[0:64, :],
                                             tg_[0:64, :])
                        nc.vector.tensor_mul(RH_B[p][64:128, :], so_[64:128, :],
                                             tg_[64:128, :])
                    else:
                        nc.vector.tensor_mul(RH_A[pn][64:128, :], so_[64:128, :],
                                             tg_[64:128, :])
                        nc.vector.tensor_mul(RH_B[pn][0:64, :], so_[0:64, :],
                                             tg_[0:64, :])

                if u == T - 1:
                    # coeffs = coeff_w @ h2 + coeff_b ; h2_19 lives in RH[pn]
                    for h_i in range(2):
                        pc = gp.tile([3, HALF], DT32, tag="gpsum", name="gpsum")
                        for c in range(NCH):
                            f = slice(c * 512, (c + 1) * 512)
                            if h_i == 0:
                                nc.tensor.matmul(pc[:, f], CTs[64:128, :],
                                                 RH_A[pn][64:128, f],
                                                 start=True, stop=True,
                                                 tile_position=(64, 0))
                            else:
                                nc.tensor.matmul(pc[:, f], CTs[0:64, :],
                                                 RH_B[pn][0:64, f],
                                                 start=True, stop=True,
                                                 tile_position=(0, 0))
                        osl = slice(h_i * HALF, (h_i + 1) * HALF)
                        nc.vector.tensor_scalar_add(coeffs_sb[:, osl],
                                                    pc[:, :], CBs[:, 0:1])
                    nc.sync.dma_start(out=coeffs_o[:, :], in_=coeffs_sb[:, :])

                if not enc:
                    s = u - T
                    # step_out = out_w @ h2_u ; h2_u in RH[pn]
                    po = gp.tile([66, HALF], DT32, tag="gpsum", name="gpsum")
                    for c in range(NCH):
                        f = slice(c * 512, (c + 1) * 512)
                        nc.tensor.matmul(po[64:66, f], OTs[64:128, :],
                                         RH_A[pn][64:128, f],
                                         start=True, stop=True,
                                         tile_position=(64, 64))
                        nc.tensor.matmul(po[0:2, f], OTs[0:64, :],
                                         RH_B[pn][0:64, f],
                                         start=True, stop=True,
                                         tile_position=(0, 0))
                    # pred = (step_out + out_b) + prev  -> x slots of XT[pn]
                    nc.vector.scalar_tensor_tensor(
                        XT[pn][64:66, 0:HALF], po[64:66, :], OBs[64:66, 0:1],
                        XT[p][64:66, 0:HALF], Alu.add, Alu.add)
                    nc.vector.scalar_tensor_tensor(
                        XT[pn][0:2, HALF:NB], po[0:2, :], OBs[0:2, 0:1],
                        XT[p][0:2, HALF:NB], Alu.add, Alu.add)
                    nc.sync.dma_start(out=decoded_o[:, s, 0:HALF],
                                      in_=XT[pn][64:66, 0:HALF])
                    nc.sync.dma_start(out=decoded_o[:, s, HALF:NB],
                                      in_=XT[pn][0:2, HALF:NB])

            # ------------------- PFM phase -------------------
            gp_cm.__exit__(None, None, None)
            sig_cm.__exit__(None, None, None)
            state_cm.__exit__(None, None, None)
            tc.strict_bb_all_engine_barrier()

            pfm = ctx.enter_context(tc.tile_pool(name="pfm", bufs=1))
            pt = ctx.enter_context(tc.tile_pool(name="pt", bufs=2))
            pp = ctx.enter_context(
                tc.tile_pool(name="pp", bufs=2, space="PSUM"))
            pq = ctx.enter_context(
                tc.tile_pool(name="pq", bufs=2, space="PSUM"))

            CUR = pfm.tile([32, BA], DT32, tag="CUR", name="CUR")
            GOALP = pfm.tile([32, BA], DT32, tag="GOALP", name="GOALP")
            DECP = pfm.tile([32, PL * BA], DT32, tag="DECP", name="DECP")
            K1D = pfm.tile([32, BA], DT32, tag="K1D", name="K1D")
            K2D = pfm.tile([32, BA], DT32, tag="K2D", name="K2D")
            KRP = pfm.tile([16, BA], DT32, tag="KRP", name="KRP")
            KRPAIR = pfm.tile([128, 2 * BA], DT32, tag="KRPAIR", name="KRPAIR")

            # cur0 = last_pos scattered to [d*16+e, ba]
            for d in range(2):
                nc.sync.dma_start(
                    out=CUR[d * 16:(d + 1) * 16, :],
                    in_=histT[d, T - 1].rearrange("(ba e) -> e ba", e=E))
                nc.sync.dma_start(
                    out=GOALP[d * 16:(d + 1) * 16, :],
                    in_=goal_d[d:d + 1, :].to_broadcast([16, BA]))
                nc.sync.dma_start(
                    out=DECP[d * 16:(d + 1) * 16, :],
                    in_=decoded_o[d].rearrange("t (ba e) -> e (t ba)", e=E))
            for dst, j in ((K1D, 0), (K2D, 1), (KRP, 2)):
                nc.sync.dma_start(
                    out=dst[0:16, :],
                    in_=coeffs_o[j].rearrange("(ba e) -> e ba", e=E))
                if dst is not KRP:
                    nc.sync.dma_start(
                        out=dst[16:32, :],
                        in_=coeffs_o[j].rearrange("(ba e) -> e ba", e=E))

            # kr replicated over e2 -> [128 pairs, 256] x2 tiles
            pkr = pp.tile([128, 2 * BA], DT32, tag="pd", name="pd")
            for tl in range(2):
                nc.tensor.matmul(pkr[:, tl * BA:(tl + 1) * BA],
                                 LREPs[:, tl * 128:(tl + 1) * 128], KRP[:, :],
                                 start=True, stop=True, tile_position=(0, 0))
            nc.scalar.copy(KRPAIR[:, :], pkr[:, :])

            for t_i in range(PL):
                # pairwise diffs via PE: [dx0|dx1|dy0|dy1] each [128, 256]
                pd_ = pp.tile([128, 4 * BA], DT32, tag="pd", name="pd")
                for k in range(4):
                    nc.tensor.matmul(pd_[:, k * BA:(k + 1) * BA],
                                     LDIFs[:, k * 128:(k + 1) * 128],
                                     CUR[:, :], start=True, stop=True,
                                     tile_position=(0, 0))
                sq = pt.tile([128, 4 * BA], DT32, tag="sq", name="sq")
                nc.scalar.activation(sq[:, :], pd_[:, :], Act.Square)
                d2 = pt.tile([128, 2 * BA], DT32, tag="d2", name="d2")
                nc.vector.tensor_add(d2[:, :], sq[:, 0:2 * BA],
                                     sq[:, 2 * BA:4 * BA])
                m2 = pt.tile([128, 2 * BA], DT32, tag="m2", name="m2")
                nc.vector.tensor_single_scalar(m2[:, :], d2[:, :], TH_LO,
                                               Alu.is_gt)
                msk = pt.tile([128, 2 * BA], DT32, tag="msk", name="msk")
                nc.vector.scalar_tensor_tensor(msk[:, :], d2[:, :], TH_HI,
                                               m2[:, :], Alu.is_lt, Alu.mult)
                s_ = pt.tile([128, 2 * BA], DT32, tag="s_", name="s_")
                nc.scalar.activation(s_[:, :], d2[:, :], Act.Sqrt)
                dist = pt.tile([128, 2 * BA], DT32, tag="dist", name="dist")
                nc.vector.tensor_single_scalar(dist[:, :], s_[:, :],
                                               float(F32(1e-6)), Alu.add)
                dsq = pt.tile([128, 2 * BA], DT32, tag="dsq", name="dsq")
                nc.vector.tensor_mul(dsq[:, :], dist[:, :], dist[:, :])
                inv = pt.tile([128, 2 * BA], DT32, tag="inv", name="inv")
                nc.vector.reciprocal(inv[:, :], dsq[:, :])
                # rep = ((kr * diff) * (1/dsq)) * mask  (xy share inv/mask)
                rep = pt.tile([128, 4 * BA], DT32, tag="rep", name="rep")
                for half in range(2):  # x part then y part
                    sl = slice(half * 2 * BA, (half + 1) * 2 * BA)
                    nc.vector.tensor_mul(rep[:, sl], pd_[:, sl], KRPAIR[:, :])
                    nc.vector.tensor_mul(rep[:, sl], rep[:, sl], inv[:, :])
                    nc.vector.tensor_mul(rep[:, sl], rep[:, sl], msk[:, :])
                # F_rep: sum over e2 -> [32, 256]
                pfr = pq.tile([32, BA], DT32, tag="ps", name="ps")
                for k in range(4):
                    # k: 0=x tile0, 1=x tile1, 2=y tile0, 3=y tile1
                    nc.tensor.matmul(pfr[:, :],
                                     LSUMs[:, k * 32:(k + 1) * 32],
                                     rep[:, k * BA:(k + 1) * BA],
                                     start=(k == 0), stop=(k == 3),
                                     tile_position=(0, 0))
                gd = pt.tile([32, BA], DT32, tag="gd", name="gd")
                nc.vector.tensor_sub(gd[:, :], GOALP[:, :], CUR[:, :])
                nc.vector.tensor_mul(gd[:, :], gd[:, :], K1D[:, :])
                pdd = pt.tile([32, BA], DT32, tag="pdd", name="pdd")
                nc.vector.tensor_sub(pdd[:, :],
                                     DECP[:, t_i * BA:(t_i + 1) * BA],
                                     CUR[:, :])
                nc.vector.tensor_mul(pdd[:, :], pdd[:, :], K2D[:, :])
                new = pt.tile([32, BA], DT32, tag="new", name="new")
                nc.vector.tensor_add(new[:, :], CUR[:, :], gd[:, :])
                nc.vector.tensor_add(new[:, :], new[:, :], pdd[:, :])
                nc.vector.tensor_add(new[:, :], new[:, :], pfr[:, :])
                if t_i == 0:
                    nc.vector.tensor_copy(CUR[:, :], new[:, :])
                else:
                    disp = pt.tile([32, BA], DT32, tag="disp", name="disp")
                    nc.vector.tensor_sub(disp[:, :], new[:, :], CUR[:, :])
                    dsp2 = pt.tile([32, BA], DT32, tag="dsp2", name="dsp2")
                    nc.scalar.activation(dsp2[:, :], disp[:, :], Act.Square)
                    psp = pq.tile([16, BA], DT32, tag="ps", name="ps")
                    nc.tensor.matmul(psp[:, :], LNRMs[:, :], dsp2[:, :],
                                     start=True, stop=True,
                                     tile_position=(0, 0))
                    spd = pt.tile([16, BA], DT32, tag="spd", name="spd")
                    nc.scalar.activation(spd[:, :], psp[:, :], Act.Sqrt)
                    nc.vector.tensor_single_scalar(spd[:, :], spd[:, :],
                                                   10.0, Alu.mult)
                    den = pt.tile([16, BA], DT32, tag="den", name="den")
                    nc.vector.tensor_single_scalar(den[:, :], spd[:, :],
                                                   float(F32(1e-8)), Alu.max)
                    nc.vector.reciprocal(den[:, :], den[:, :])
                    clp = pt.tile([16, BA], DT32, tag="clp", name="clp")
                    nc.vector.tensor_scalar(clp[:, :], spd[:, :],
                                            float(MIN_SP), float(MAX_SP),
                                            Alu.max, Alu.min)
                    nc.vector.tensor_mul(clp[:, :], clp[:, :], den[:, :])
                    psc = pq.tile([32, BA], DT32, tag="ps", name="ps")
                    nc.tensor.matmul(psc[:, :], LDUPs[:, :], clp[:, :],
                                     start=True, stop=True,
                                     tile_position=(0, 0))
                    nc.vector.tensor_mul(disp[:, :], disp[:, :], psc[:, :])
                    nc.vector.tensor_add(CUR[:, :], CUR[:, :], disp[:, :])
                nc.sync.dma_start(out=adjusted_o[t_i, :, :], in_=CUR[:, :])

    return nc


# ---------------------------------------------------------------------------
# host side
# ---------------------------------------------------------------------------

def _lhsT_gates(Wmat):
    """[256, K] weight -> lhsT [K, 256] (gate-major columns)."""
    return np.ascontiguousarray(Wmat.T).astype(F32)


def _prep_consts(params):
    f = lambda x: np.asarray(x, F32)
    emb_w, emb_b = f(params["emb_w"]), f(params["emb_b"])
    out = {}

    def fuse1(layer):
        Wih, Whh, b = f(layer["Wih"]), f(layer["Whh"]), f(layer["b"])
        Wx = f(Wih @ emb_w)            # [256, 2]
        be = f(b + Wih @ emb_b)        # [256]
        return Whh, Wx, be

    def pack_lhsT_h(Whh):
        # [128, 256]: rows 0-63 and 64-127 both = Whh.T (for half A / B)
        lt = _lhsT_gates(Whh)          # [64, 256]
        return np.concatenate([lt, lt], axis=0)

    def pack_lhsT_x(Wx):
        # [66, 256]: rows 0-1 (half B) and 64-65 (half A) = Wx.T
        lt = _lhsT_gates(Wx)           # [2, 256]
        arr = np.zeros((66, 256), F32)
        arr[0:2] = lt
        arr[64:66] = lt
        return arr

    def pack_l2(Wih, Whh):
        # A: K-order [h1; h2]; B: K-order [h2; h1]
        a = np.concatenate([_lhsT_gates(f(Wih)), _lhsT_gates(f(Whh))], axis=0)
        b_ = np.concatenate([_lhsT_gates(f(Whh)), _lhsT_gates(f(Wih))], axis=0)
        return a, b_

    def pack_bias(be):
        # [128, 4]: per-gate column, duplicated for both halves
        arr = np.zeros((128, 4), F32)
        for g in range(4):
            arr[0:64, g] = be[g * 64:(g + 1) * 64]
            arr[64:128, g] = be[g * 64:(g + 1) * 64]
        return arr

    W1h, W1x, b1 = fuse1(params["enc"][0])
    out["WL1"] = pack_lhsT_h(W1h)
    out["WX1"] = pack_lhsT_x(W1x)
    out["BE1"] = pack_bias(b1)
    out["WL2A"], out["WL2B"] = pack_l2(params["enc"][1]["Wih"],
                                       params["enc"][1]["Whh"])
    out["BE2"] = pack_bias(f(params["enc"][1]["b"]))
    V1h, V1x, c1 = fuse1(params["dec"][0])
    out["VL1"] = pack_lhsT_h(V1h)
    out["VX1"] = pack_lhsT_x(V1x)
    out["BD1"] = pack_bias(c1)
    out["VL2A"], out["VL2B"] = pack_l2(params["dec"][1]["Wih"],
                                       params["dec"][1]["Whh"])
    out["BD2"] = pack_bias(f(params["dec"][1]["b"]))

    coeff_w, coeff_b = f(params["coeff_w"]), f(params["coeff_b"])
    ct = np.zeros((128, 3), F32)
    ct[0:64] = coeff_w.T
    ct[64:128] = coeff_w.T
    out["CT"] = ct
    out["CB"] = coeff_b.reshape(3, 1).astype(F32)
    out_w, out_b = f(params["out_w"]), f(params["out_b"])
    ot = np.zeros((128, 2), F32)
    ot[0:64] = out_w.T
    ot[64:128] = out_w.T
    out["OT"] = ot
    ob = np.zeros((66, 1), F32)
    ob[0:2, 0] = out_b
    ob[64:66, 0] = out_b
    out["OB"] = ob

    # PFM matmul constants
    ldif = np.zeros((32, 512), F32)   # [dx0|dx1|dy0|dy1], each [32, 128]
    for tl in range(2):
        for r in range(128):
            e1, e2 = (r + tl * 128) // E, (r + tl * 128) % E
            ldif[0:16, tl * 128 + r] = 0.0
            ldif[e1, tl * 128 + r] += 1.0
            ldif[e2, tl * 128 + r] -= 1.0
            ldif[16 + e1, 256 + tl * 128 + r] += 1.0
            ldif[16 + e2, 256 + tl * 128 + r] -= 1.0
    out["LDIF"] = ldif
    lsum = np.zeros((128, 128), F32)  # [sx0|sx1|sy0|sy1], each [128, 32]
    for k in range(4):
        tl = k % 2
        coord = k // 2
        for r in range(128):
            e1 = (r + tl * 128) // E
            lsum[r, k * 32 + coord * 16 + e1] = 1.0
    out["LSUM"] = lsum
    lrep = np.zeros((16, 256), F32)   # [r0|r1], each [16, 128]
    for tl in range(2):
        for r in range(128):
            e1 = (r + tl * 128) // E
            lrep[e1, tl * 128 + r] = 1.0
    out["LREP"] = lrep
    eye16 = np.eye(16, dtype=F32)
    out["LNRM"] = np.concatenate([eye16, eye16], axis=0)   # [32, 16]
    out["LDUP"] = np.concatenate([eye16, eye16], axis=1)   # [16, 32]
    return out


_PROGRAM = None


def _get_program():
    global _PROGRAM
    if _PROGRAM is None:
        _PROGRAM = _build_program()
    return _PROGRAM


def kernel(history_neighbors, goal, params):
    _install_ntff_hook()
    from concourse.bass_utils import run_bass_kernel_spmd

    nc = _get_program()
    consts = _prep_consts(params)

    hist = np.asarray(history_neighbors, F32)
    goal = np.asarray(goal, F32)

    in_maps = []
    for c in range(NCORES):
        hc = hist[c * BC:(c + 1) * BC].reshape(NB, T, D)
        histT_c = np.ascontiguousarray(hc.transpose(2, 1, 0))   # [2, 20, NB]
        gc = goal[c * BC:(c + 1) * BC].reshape(BA, D)
        goal_c = np.ascontiguousarray(gc.T)                     # [2, BA]
        m = {"histT": histT_c, "goal_d": goal_c}
        m.update(consts)
        in_maps.append(m)

    res = run_bass_kernel_spmd(nc, in_maps, core_ids=list(range(NCORES)))

    adjusted = np.zeros((B, A, E, PL, D), F32)
    decoded = np.zeros((B, A, E, PL, D), F32)
    coeff_mean = np.zeros((B, A, E, 3), F32)
    for c in range(NCORES):
        r = res.results[c]
        sl = slice(c * BC, (c + 1) * BC)
        # adjusted_o [12, 32=(d,e), 256=(b,a)]
        adjusted[sl] = r["adjusted_o"].reshape(PL, D, E, BC, A).transpose(
            3, 4, 2, 0, 1)
        decoded[sl] = r["decoded_o"].reshape(D, PL, BC, A, E).transpose(
            2, 3, 4, 1, 0)
        coeff_mean[sl] = r["coeffs_o"].reshape(3, BC, A, E).transpose(
            1, 2, 3, 0)
    return adjusted, decoded, coeff_mean, np.zeros_like(coeff_mean)
